# revision 4
# baseline (speedup 1.0000x reference)
"""Trainium2 Bass kernel for ContextualBiasAttention (B=2, S=2048, H=1024, nh=16).

Sharding: tensor-parallel over heads -- each of the 8 cores owns 2 heads.
Wq/Wk/Wv column-sharded, Wo row-sharded; o_proj partials summed on host.

Device layout per core (heads h0,h1 on partitions 0-63 / 64-127):
  qT/kT channel-major [128, 2048] per batch (RoPE applied on-chip),
  v token-major [128 t, 128 c] chunks (PE transpose of channel-major proj),
  scores^T blocks [128 k, 512 q] = K-block-stationary matmul, bias+causal mask
  added via identity-matmul of an fp16 bias (mask = -60000 -> exp == 0),
  exp on ACT (no max subtraction: |scores| <= ~10), rowsum via ones-matmul,
  normalize with reciprocal broadcast (K=1 matmul), attn@v on unnormalized
  exp values scaled afterwards.  All PE inputs are float32r (~2^-14 rounding).
Causal block structure: for q-tile j (512 wide) only k-tiles 0..4j+3 computed;
untouched attn output regions return as zeros (outputs are pre-zeroed).
"""
import sys

sys.path.insert(0, "/opt/trn_rl_repo")

import math

import numpy as np

import concourse.bass as bass
import concourse.mybir as mybir
from concourse import bacc, tile
from concourse.bass_utils import run_bass_kernel_spmd

F32 = mybir.dt.float32
F32R = mybir.dt.float32r
F16 = mybir.dt.float16

B = 2
S = 2048
H = 1024
NH = 16
HD = 64
NCORES = 8
HPC = NH // NCORES          # heads per core = 2
CPC = HPC * HD              # channels per core = 128
NJ = S // 512               # q tiles of 512 -> 4
NI = S // 128               # k tiles of 128 -> 16
NI4 = S // 512              # 512-token tiles for projections -> 4
MASK = -60000.0             # fp16-representable; exp() underflows to exactly 0
ROPE_BASE = 10000.0

_PROG = None                # cached (nc, input_names)


def _build_program():
    nc = bacc.Bacc()

    xT = nc.declare_dram_parameter("xT", [B, H, S], F32R, isOutput=False)
    wqT = nc.declare_dram_parameter("wqT", [H, CPC], F32R, isOutput=False)
    wkT = nc.declare_dram_parameter("wkT", [H, CPC], F32R, isOutput=False)
    wvT = nc.declare_dram_parameter("wvT", [H, CPC], F32R, isOutput=False)
    woT = nc.declare_dram_parameter("woT", [CPC, H], F32R, isOutput=False)
    biasMT = nc.declare_dram_parameter("biasMT", [B, S, S], F16, isOutput=False)
    cos2 = nc.declare_dram_parameter("cos2", [128, S], F32, isOutput=False)
    sin2 = nc.declare_dram_parameter("sin2", [128, S], F32, isOutput=False)
    rotm = nc.declare_dram_parameter("rotm", [128, 128], F32R, isOutput=False)
    identR = nc.declare_dram_parameter("identR", [128, 128], F32R, isOutput=False)
    identH = nc.declare_dram_parameter("identH", [128, 128], F16, isOutput=False)
    ones128 = nc.declare_dram_parameter("ones128", [128, 1], F32R, isOutput=False)
    ones1 = nc.declare_dram_parameter("ones1", [1, 128], F32R, isOutput=False)

    attnT = nc.declare_dram_parameter("attnT", [B, HPC, S, S], F32, isOutput=True)
    outp = nc.declare_dram_parameter("outp", [B, S, H], F32, isOutput=True)

    with tile.TileContext(nc) as tc:
        with (
            tc.tile_pool(name="const", bufs=1) as cpool,
            tc.tile_pool(name="wpool", bufs=1) as wpool,
            tc.tile_pool(name="qk", bufs=1) as qkpool,
            tc.tile_pool(name="vres", bufs=1) as vpool,
            tc.tile_pool(name="xt", bufs=3) as xtpool,
            tc.tile_pool(name="work", bufs=2) as work,
            tc.tile_pool(name="psbp", bufs=9) as psbp,
            tc.tile_pool(name="asbp", bufs=3) as asbp,
            tc.tile_pool(name="biasp", bufs=1) as biasp,
            tc.tile_pool(name="mm2", bufs=2, space="PSUM") as mm2,
            tc.tile_pool(name="accp", bufs=1, space="PSUM") as accp,
            tc.tile_pool(name="auxp", bufs=2, space="PSUM") as auxp,
        ):
            # ---- constants ----
            cossb = cpool.tile([128, S], F32, name="cossb")
            sinsb = cpool.tile([128, S], F32, name="sinsb")
            rotsb = cpool.tile([128, 128], F32R, name="rotsb")
            idRsb = cpool.tile([128, 128], F32R, name="idRsb")
            idHsb = cpool.tile([128, 128], F16, name="idHsb")
            o128 = cpool.tile([128, 1], F32R, name="o128")
            o1 = cpool.tile([1, 128], F32R, name="o1")
            nc.sync.dma_start(cossb[:], cos2[:])
            nc.sync.dma_start(sinsb[:], sin2[:])
            nc.sync.dma_start(rotsb[:], rotm[:])
            nc.sync.dma_start(idRsb[:], identR[:])
            nc.sync.dma_start(idHsb[:], identH[:])
            nc.sync.dma_start(o128[:], ones128[:])
            nc.sync.dma_start(o1[:], ones1[:])

            # ---- weights (lhsT tiles [128 i, c]) ----
            wq = wpool.tile([128, 8, CPC], F32R, name="wq")
            wk = wpool.tile([128, 8, CPC], F32R, name="wk")
            wv = wpool.tile([128, 8, CPC], F32R, name="wv")
            wo = wpool.tile([CPC, H], F32R, name="wo")
            nc.sync.dma_start(wq[:], wqT.rearrange("(i p) c -> p i c", p=128))
            nc.sync.dma_start(wk[:], wkT.rearrange("(i p) c -> p i c", p=128))
            nc.sync.dma_start(wv[:], wvT.rearrange("(i p) c -> p i c", p=128))
            nc.sync.dma_start(wo[:], woT[:])

            # ---- per-batch residents ----
            qTs = [qkpool.tile([128, S], F32R, name=f"qTs{b}") for b in range(B)]
            kTs = [qkpool.tile([128, S], F32R, name=f"kTs{b}") for b in range(B)]
            vSB = [vpool.tile([128, S], F32R, name=f"vSB{b}") for b in range(B)]

            # =========================== projections ===========================
            for b in range(B):
                for j4 in range(NI4):
                    qkPS = mm2.tile([128, 1024], F32, name=f"qk_{b}_{j4}", tag="mm2")
                    vPS = auxp.tile([128, 512], F32, name=f"vps_{b}_{j4}", tag="aux")
                    for i in range(8):
                        xsl = xtpool.tile(
                            [128, 512], F32R, name=f"x_{b}_{j4}_{i}", tag="xt"
                        )
                        nc.sync.dma_start(
                            xsl[:],
                            xT[b, i * 128 : (i + 1) * 128, j4 * 512 : (j4 + 1) * 512],
                        )
                        nc.tensor.matmul(
                            qkPS[:, 0:512], wq[:, i, :], xsl[:],
                            start=(i == 0), stop=(i == 7),
                        )
                        nc.tensor.matmul(
                            qkPS[:, 512:1024], wk[:, i, :], xsl[:],
                            start=(i == 0), stop=(i == 7),
                        )
                        nc.tensor.matmul(
                            vPS[:], wv[:, i, :], xsl[:],
                            start=(i == 0), stop=(i == 7),
                        )
                    # raw q/k to SBUF (fp32r) for the rotation matmul
                    qkraw = work.tile([128, 1024], F32R, name=f"qkraw_{b}_{j4}", tag="qkraw")
                    with nc.allow_low_precision(reason="f32r rounding for PE"):
                        nc.scalar.copy(qkraw[:], qkPS[:])
                    # v channel-major -> SBUF, then PE-transpose to token-major
                    vcm = work.tile([128, 512], F32R, name=f"vcm_{b}_{j4}", tag="vcm")
                    with nc.allow_low_precision(reason="f32r rounding for PE"):
                        nc.scalar.copy(vcm[:], vPS[:])
                    vtr = auxp.tile([128, 512], F32R, name=f"vtr_{b}_{j4}", tag="aux")
                    for t in range(4):
                        nc.tensor.transpose(
                            vtr[:, t * 128 : (t + 1) * 128],
                            vcm[:, t * 128 : (t + 1) * 128],
                            idRsb[:],
                        )
                    with nc.allow_low_precision(reason="f32r rounding for PE"):
                        nc.vector.tensor_copy(
                            vSB[b][:, j4 * 512 : (j4 + 1) * 512], vtr[:]
                        )
                    # RoPE: q' = q*cos + (R q)*sin   (R = signed half-rotation)
                    for s, dst in ((0, qTs[b]), (1, kTs[b])):
                        src = qkraw[:, 512 * s : 512 * s + 512]
                        rotPS = auxp.tile(
                            [128, 512], F32, name=f"rot_{b}_{j4}_{s}", tag="aux"
                        )
                        nc.tensor.matmul(rotPS[:], rotsb[:], src, start=True, stop=True)
                        csl = slice(j4 * 512, (j4 + 1) * 512)
                        t1 = work.tile([128, 512], F32, name=f"t1_{b}_{j4}_{s}", tag="t1")
                        nc.vector.tensor_mul(t1[:], src.bitcast(F32), cossb[:, csl])
                        t2 = work.tile([128, 512], F32, name=f"t2_{b}_{j4}_{s}", tag="t2")
                        nc.vector.tensor_mul(t2[:], rotPS[:], sinsb[:, csl])
                        with nc.allow_low_precision(reason="f32r rounding for PE"):
                            nc.vector.tensor_add(dst[:, csl], t1[:], t2[:])

            # =========================== attention =============================
            for j in range(NJ):
                nI = 4 * (j + 1)          # causal k-tiles for this q tile
                qsl = slice(j * 512, (j + 1) * 512)
                for b in range(B):
                    # stage the fp16 bias blocks for (j, b); reused by both heads
                    bias_t = []
                    for i in range(nI):
                        bt = biasp.tile(
                            [128, 512], F16, name=f"bias_{j}_{b}_{i}", tag=f"bias{i}"
                        )
                        nc.sync.dma_start(
                            bt[:], biasMT[b, i * 128 : (i + 1) * 128, qsl]
                        )
                        bias_t.append(bt)

                    aosb = asbp.tile([128, 512], F32R, name=f"aosb_{j}_{b}", tag="aosb", bufs=2)
                    for h in range(HPC):
                        hsl = slice(64 * h, 64 * h + 64)
                        rsPS = accp.tile([1, 512], F32, name=f"rs_{j}_{b}_{h}", tag="rs")
                        aoPS = accp.tile([64, 512], F32, name=f"ao_{j}_{b}_{h}", tag="ao")
                        psbs = []
                        for ip in range(nI // 2):
                            sPS = mm2.tile(
                                [128, 1024], F32, name=f"s_{j}_{b}_{h}_{ip}", tag="mm2"
                            )
                            for s2 in range(2):
                                i = 2 * ip + s2
                                ssl = slice(512 * s2, 512 * s2 + 512)
                                nc.tensor.matmul(
                                    sPS[:, ssl],
                                    kTs[b][hsl, i * 128 : (i + 1) * 128],
                                    qTs[b][hsl, qsl],
                                    start=True, stop=False,
                                )
                                with nc.allow_low_precision(
                                    reason="identity@bias exact; psum accum fp32"
                                ):
                                    nc.tensor.matmul(
                                        sPS[:, ssl], idHsb[:], bias_t[i][:],
                                        start=False, stop=True,
                                    )
                            psb = psbp.tile(
                                [128, 1024], F32R, name=f"p_{j}_{b}_{h}_{ip}", tag="psb"
                            )
                            with nc.allow_low_precision(reason="f32r exp for PE"):
                                nc.scalar.activation(
                                    psb[:], sPS[:], mybir.ActivationFunctionType.Exp
                                )
                            psbs.append(psb)
                            for s2 in range(2):
                                i = 2 * ip + s2
                                ssl = slice(512 * s2, 512 * s2 + 512)
                                nc.tensor.matmul(
                                    rsPS[:], o128[:], psb[:, ssl],
                                    start=(i == 0), stop=(i == nI - 1),
                                )
                                nc.tensor.matmul(
                                    aoPS[:],
                                    vSB[b][:, i * 128 + 64 * h : i * 128 + 64 * h + 64],
                                    psb[:, ssl],
                                    start=(i == 0), stop=(i == nI - 1),
                                )
                        # reciprocal of rowsums, broadcast to 128 partitions
                        rsb = work.tile([1, 512], F32R, name=f"r_{j}_{b}_{h}", tag="rsb")
                        with nc.allow_low_precision(reason="f32r recip for PE bcast"):
                            nc.vector.reciprocal(rsb[:], rsPS[:])
                        rbcPS = auxp.tile(
                            [128, 512], F32, name=f"rbc_{j}_{b}_{h}", tag="aux"
                        )
                        nc.tensor.matmul(rbcPS[:], o1[:], rsb[:], start=True, stop=True)
                        rbcsb = work.tile(
                            [128, 512], F32, name=f"rbcs_{j}_{b}_{h}", tag="rbcs"
                        )
                        nc.scalar.copy(rbcsb[:], rbcPS[:])
                        # normalized attention blocks -> DRAM (split DVE/GPSIMD)
                        for i in range(nI):
                            psb = psbs[i // 2]
                            ssl = slice(512 * (i % 2), 512 * (i % 2) + 512)
                            asb = asbp.tile(
                                [128, 512], F32, name=f"a_{j}_{b}_{h}_{i}", tag="asb"
                            )
                            eng = nc.gpsimd if (i % 3 == 2) else nc.vector
                            eng.tensor_mul(
                                asb[:], psb[:, ssl].bitcast(F32), rbcsb[:]
                            )
                            nc.sync.dma_start(
                                attnT[b, h, i * 128 : (i + 1) * 128, qsl], asb[:]
                            )
                        # attn@v output, normalized, into combined [128,512] tile
                        aocp = work.tile([64, 512], F32, name=f"aoc_{j}_{b}_{h}", tag="aoc")
                        nc.scalar.copy(aocp[:], aoPS[:])
                        with nc.allow_low_precision(reason="f32r rounding for PE"):
                            nc.vector.tensor_mul(
                                aosb[hsl, :], aocp[:], rbcsb[0:64, :]
                            )
                    # o_proj for (b, j): out[t, m] partial
                    for q1 in range(4):
                        osl = slice(q1 * 128, q1 * 128 + 128)
                        osb = work.tile([128, 1024], F32, name=f"os_{j}_{b}_{q1}", tag="osb")
                        for m in range(2):
                            oPS = auxp.tile(
                                [128, 512], F32, name=f"o_{j}_{b}_{q1}_{m}", tag="aux"
                            )
                            nc.tensor.matmul(
                                oPS[:],
                                aosb[:, osl],
                                wo[:, m * 512 : (m + 1) * 512],
                                start=True, stop=True,
                            )
                            if m == 0:
                                nc.scalar.copy(osb[:, 0:512], oPS[:])
                            else:
                                nc.vector.tensor_copy(osb[:, 512:1024], oPS[:])
                        nc.sync.dma_start(
                            outp[b, j * 512 + q1 * 128 : j * 512 + (q1 + 1) * 128, :],
                            osb[:],
                        )

    nc.finalize()
    return nc


def _host_prep(x, attention_mask, Wq, Wk, Wv, Wo, contextual_bias):
    """Build shared + per-core device input arrays."""
    xT = np.ascontiguousarray(x.transpose(0, 2, 1)).astype(np.float32)

    # RoPE tables, transposed to [hd, S], stacked for 2 heads
    inv = 1.0 / (ROPE_BASE ** (np.arange(0, HD, 2, dtype=np.float32) / HD))
    t = np.arange(S, dtype=np.float32)
    freqs = np.outer(t, inv)                       # [S, 32]
    emb = np.concatenate([freqs, freqs], axis=-1)  # [S, 64]
    cosT = np.cos(emb).T.astype(np.float32)        # [64, S]
    sinT = np.sin(emb).T.astype(np.float32)
    cos2 = np.vstack([cosT, cosT])                 # [128, S]
    sin2 = np.vstack([sinT, sinT])

    # signed half-rotation: rot[d'] = -q[d'+32] (d'<32) ; +q[d'-32] (d'>=32)
    # lhsT[d, d'] so that (lhsT.T @ q)[d'] = rot[d']
    R = np.zeros((128, 128), np.float32)
    for h in range(2):
        o = 64 * h
        for d in range(32):
            R[o + d + 32, o + d] = -1.0
            R[o + d, o + d + 32] = 1.0

    # bias + causal mask (+ padding mask), fp16, transposed [k, q] per batch
    causal = np.triu(np.ones((S, S), bool), 1)
    biasM = np.where(causal, np.float32(MASK), contextual_bias.astype(np.float32))
    biasMT = np.empty((B, S, S), np.float16)
    for b in range(B):
        mb = biasM if attention_mask[b].all() else np.where(
            (attention_mask[b] == 0)[None, :], np.float32(MASK), biasM
        )
        biasMT[b] = mb.T.astype(np.float16)

    shared = {
        "xT": xT,
        "biasMT": biasMT,
        "cos2": cos2,
        "sin2": sin2,
        "rotm": R,
        "identR": np.eye(128, dtype=np.float32),
        "identH": np.eye(128, dtype=np.float16),
        "ones128": np.ones((128, 1), np.float32),
        "ones1": np.ones((1, 128), np.float32),
    }
    in_maps = []
    for c in range(NCORES):
        hs, he = c * CPC, (c + 1) * CPC
        m = dict(shared)
        m["wqT"] = np.ascontiguousarray((Wq[hs:he, :] / 8.0).T).astype(np.float32)
        m["wkT"] = np.ascontiguousarray(Wk[hs:he, :].T).astype(np.float32)
        m["wvT"] = np.ascontiguousarray(Wv[hs:he, :].T).astype(np.float32)
        m["woT"] = np.ascontiguousarray(Wo[:, hs:he].T).astype(np.float32)
        in_maps.append(m)
    return in_maps


def kernel(x, attention_mask, Wq, Wk, Wv, Wo, contextual_bias):
    global _PROG
    x = np.asarray(x, np.float32)
    attention_mask = np.asarray(attention_mask)
    if _PROG is None:
        _PROG = _build_program()
    in_maps = _host_prep(
        x, attention_mask,
        np.asarray(Wq, np.float32), np.asarray(Wk, np.float32),
        np.asarray(Wv, np.float32), np.asarray(Wo, np.float32),
        np.asarray(contextual_bias, np.float32),
    )
    res = run_bass_kernel_spmd(_PROG, in_maps, list(range(NCORES))).results

    out = np.zeros((B, S, H), np.float32)
    for c in range(NCORES):
        out += res[c]["outp"]
    attn = np.empty((B, NH, S, S), np.float32)
    for c in range(NCORES):
        a = res[c]["attnT"]                      # [B, HPC, k, q]
        for b in range(B):
            for hl in range(HPC):
                attn[b, HPC * c + hl] = a[b, hl].T
    return out, attn
